# revision 1
# baseline (speedup 1.0000x reference)
"""Trainium2 Bass kernel for nn_CrossAttention_14207751815513.

Single-query cross-attention:
    q = x1 @ Wq.T                 (one query per head)
    k = x2 @ Wk.T ; v = x2 @ Wv.T
    attn_h = softmax(q_h . k_h / sqrt(128))
    out = concat_h(attn_h @ v_h) @ Wo.T + bo

Because there is exactly ONE query, the K and V projections collapse
algebraically (associativity):
    scores_h = x2 @ r_h,  r_h = Wk_h.T q_h / sqrt(128)   -- no k materialization
    out_h    = Wv_h @ (x2.T p_h) / l_h                   -- no v materialization
with p = exp(scores) (logits are small, |s| < ~6, so no max subtraction
is needed) and l_h = sum_s p_h[s].

Sharding: the sequence dim (16384) is split across the 8 NeuronCores
(2048 rows each).  Every quantity that is O(1) in the sequence length
(q, R = [r_1..r_16], the per-head Wv matvec, Wo + bias) lives in the
host-side shard-prep / gather-merge glue; the O(S*C) work runs on
device.  Per-core device program (matmuls bf16, fp32 PSUM):

  S  : scores[h, s] = sum_c R[c, h] x2t[c, s]     (16 K-chunks x 4 banks)
  exp: P = exp(scores)                            (ScalarE LUT)
  tr : P [16, 2048] -> PT [128, 16 sb, 16 h]      (PE transposes)
  L  : l = P @ ones   (bf16-consistent row sums)
  T  : t^T[h, c] = sum_s PT[s, h] x2n[s, c]

Outputs per core: tt [16, 2048] f32 (unnormalized x2^T p), l [16, 1].
Host merge: sum partials over cores, normalize by l, apply Wv per head,
then Wo + bo.

Sync-wait note: this backend disables DynamicDMA, so every HW-DGE DMA
lowers to a pseudo-direct DMA that supports at most ONE semaphore wait
("Too many sync wait commands" in walrus codegen otherwise).  The
program is therefore structured so no DMACopy ever needs two waits:
  - every streamed tile is a fresh buffer (unique pool tag, no reuse)
    so stream DMAs carry no WAR/WAW waits;
  - the program issues exactly 8 DMAs total (the 8 HW-DGE semaphore
    slots are assigned globally round-robin across both rings), so no
    DMA ever carries a slot-recycle wait on top of a RAW wait: rsb +
    3 coarse x2t chunks on the SP ring, 3 coarse x2n chunks on the
    Act ring, and one merged output DMA (l folded into tt's last
    column, all its producers on the scalar engine);
  - the end-of-context Drain gets a sem wait for every proc the SP
    engine hasn't directly observed (the wait clock is not
    transitive), so an epilogue of single-dep SP nops makes SP
    observe each DMA and each engine's last instruction first.
"""

import sys

for _p in ("/root/.axon_site/_ro/trn_rl_repo", "/opt/trn_rl_repo"):
    if _p not in sys.path:
        sys.path.append(_p)

import numpy as np
import ml_dtypes

import concourse.bass as bass
import concourse.tile as tile
from concourse import mybir
from concourse.bass_utils import run_bass_kernel_spmd
from concourse.masks import make_identity
from concourse.tile_rust import add_dep_helper

NCORES = 8
S_FULL = 16384
C = 2048           # input feature dim (both x1 and x2)
H = 16             # heads
J = 128            # head dim (K_DIM == V_DIM == 128)
HJ = H * J         # 2048
ODIM = 512
S_LOC = S_FULL // NCORES   # 2048 sequence rows per core

BF = mybir.dt.bfloat16
F32 = mybir.dt.float32
INV_SQRT_K = 1.0 / float(np.sqrt(128.0))

NB = 512                    # PSUM bank free-dim (f32 columns)
CH = C // 128               # 16 chunks of 128 along any 2048 dim

_BF_NP = ml_dtypes.bfloat16


def _build_program() -> bass.Bass:
    nc = bass.Bass()
    # x2t/x2n are packed partition-major on the host ([p, chunk, col]) so a
    # multi-chunk stream DMA folds to ONE contiguous descriptor per partition.
    t_in = {
        "rsb": nc.dram_tensor("rsb", [J, CH, H], BF, kind="ExternalInput"),
        "x2t": nc.dram_tensor("x2t", [J, CH, S_LOC], BF, kind="ExternalInput"),
        "x2n": nc.dram_tensor("x2n", [J, CH, C], BF, kind="ExternalInput"),
    }
    t_out = {
        "tt": nc.dram_tensor("tt", [H, C + 1], F32, kind="ExternalOutput"),
    }

    rsb_d = t_in["rsb"][:, :, :]
    x2t_v = t_in["x2t"][:, :, :]
    x2n_v = t_in["x2n"][:, :, :]
    tt_out = t_out["tt"][:, :]

    # Coarse stream DMAs (8 DMAs total incl. rsb + output), issued at the
    # top of the program and split across both HW-DGE rings so each ring
    # streams ~8MB: SP carries S chunks cc0/cc1-6 then T chunk sc0-7; Act
    # carries rsb, S chunk cc7-15, T chunk sc8-15, and the output.  First S
    # chunk is small for an early matmul start; per-ring byte order matches
    # PE consumption order.
    S_CHUNKS_SP = [(0, 1), (1, 6)]
    S_CHUNKS_ACT = [(7, 9)]
    T_CHUNKS_SP = [(0, 8)]
    T_CHUNKS_ACT = [(8, 8)]

    with tile.TileContext(nc) as tc:
        with (
            tc.tile_pool(name="singles", bufs=1) as singles,
            tc.tile_pool(name="sa", bufs=1) as sa,
            tc.tile_pool(name="sb", bufs=1) as sbp,
            tc.tile_pool(name="psbig", bufs=4, space="PSUM") as psbig,
            tc.tile_pool(name="pstr", bufs=2, space="PSUM") as pstr,
            tc.tile_pool(name="psl", bufs=1, space="PSUM") as pslp,
        ):
            ep_targets = []  # one representative instruction per proc

            # ---- issue every stream DMA up front (both rings) --------------
            Rsb = singles.tile([J, CH, H], BF)
            ep_targets.append(nc.scalar.dma_start(out=Rsb, in_=rsb_d))

            s_tiles = []        # (first_cc, n_cc, tile)
            for ci, (a, k) in enumerate(S_CHUNKS_SP):
                xt = sa.tile([128, k, S_LOC], BF, tag=f"sa_sp{ci}")
                ep_targets.append(nc.sync.dma_start(out=xt, in_=x2t_v[:, a : a + k, :]))
                s_tiles.append((a, k, xt))
            for ci, (a, k) in enumerate(S_CHUNKS_ACT):
                xt = sa.tile([128, k, S_LOC], BF, tag=f"sa_act{ci}")
                ep_targets.append(nc.scalar.dma_start(out=xt, in_=x2t_v[:, a : a + k, :]))
                s_tiles.append((a, k, xt))
            t_tiles = []
            for ci, (a, k) in enumerate(T_CHUNKS_SP):
                xn = sbp.tile([128, k, C], BF, tag=f"sb_sp{ci}")
                ep_targets.append(nc.sync.dma_start(out=xn, in_=x2n_v[:, a : a + k, :]))
                t_tiles.append((a, k, xn))
            for ci, (a, k) in enumerate(T_CHUNKS_ACT):
                xn = sbp.tile([128, k, C], BF, tag=f"sb_act{ci}")
                ep_targets.append(nc.scalar.dma_start(out=xn, in_=x2n_v[:, a : a + k, :]))
                t_tiles.append((a, k, xn))

            ident16 = singles.tile([16, 16], BF)
            nc.gpsimd.memset(ident16, 0.0)
            # out[x, y] = (x - y) != 0 ? 0.0 : 1.0  (make_identity, inlined
            # to keep the instruction handle for the drain funnel)
            i_pool = nc.gpsimd.affine_select(
                out=ident16,
                in_=ident16,
                compare_op=mybir.AluOpType.not_equal,
                fill=1.0,
                base=0,
                pattern=[[-1, 16]],
                channel_multiplier=1,
            )
            ones_bf = singles.tile([J, 1], BF)
            nc.vector.memset(ones_bf, 1.0)

            # ---- phase S: scores[h, s] = sum_c R[c, h] x2t[c, s] -----------
            ps_s = [
                psbig.tile([16, NB], F32, tag="big", name=f"ps_s{m}")
                for m in range(4)
            ]
            for a, k, xt in s_tiles:
                for g in range(k):
                    cc = a + g
                    for m in range(4):
                        nc.tensor.matmul(
                            ps_s[m][:16, :],
                            lhsT=Rsb[:, cc, :],
                            rhs=xt[:, g, m * NB : (m + 1) * NB],
                            start=(cc == 0),
                            stop=(cc == CH - 1),
                        )

            # ---- exp (no max subtraction needed; |logit| < ~6) -------------
            Psb = singles.tile([16, S_LOC], BF)
            for m in range(4):
                nc.scalar.activation(
                    out=Psb[:, m * NB : (m + 1) * NB],
                    in_=ps_s[m][:16, :],
                    func=mybir.ActivationFunctionType.Exp,
                )

            # ---- transpose P -> PT [128, 16 schunk, 16 h] ------------------
            PT = singles.tile([J, CH, H], BF)
            for sb in range(CH):
                ps = pstr.tile([J, H], BF, tag="tr")
                nc.tensor.transpose(ps, Psb[:, sb * 128 : (sb + 1) * 128], ident16)
                i_dve = nc.vector.tensor_copy(out=PT[:, sb, :], in_=ps)

            # ---- phase L: l[h] = sum_s P[h, s] (bf16-consistent) -----------
            ps_l = pslp.tile([16, 1], F32, tag="l")
            for sb in range(CH):
                nc.tensor.matmul(
                    ps_l,
                    lhsT=PT[:, sb, :],
                    rhs=ones_bf,
                    start=(sb == 0),
                    stop=(sb == CH - 1),
                )
            # ---- phase T: t^T[h, c] = sum_s PT[s, h] x2n[s, c] -------------
            ps_t = [
                psbig.tile([16, NB], F32, tag="big", name=f"ps_t{m}")
                for m in range(4)
            ]
            for a, k, xn in t_tiles:
                for g in range(k):
                    sc = a + g
                    for m in range(4):
                        i_pe = nc.tensor.matmul(
                            ps_t[m][:16, :],
                            lhsT=PT[:, sc, :],
                            rhs=xn[:, g, m * NB : (m + 1) * NB],
                            start=(sc == 0),
                            stop=(sc == CH - 1),
                        )
            tt_sb = singles.tile([16, C + 1], F32)
            i_copies = [nc.scalar.copy(out=tt_sb[:, C : C + 1], in_=ps_l)]
            for m in range(4):
                i_copies.append(
                    nc.scalar.copy(
                        out=tt_sb[:, m * NB : (m + 1) * NB], in_=ps_t[m][:16, :]
                    )
                )
            i_out = nc.scalar.dma_start(out=tt_out, in_=tt_sb)

            # ---- drain-funnel epilogue (see sync-wait note above) ----------
            ep_targets += [i_pool, i_dve, i_pe, *i_copies, i_out]
            for t in ep_targets:
                n = nc.sync.nop(nofuse=True, hint="dep")
                add_dep_helper(n.ins, t.ins, reason="drain-funnel")

    return nc


_NC_CACHE = None


def _get_nc() -> bass.Bass:
    global _NC_CACHE
    if _NC_CACHE is None:
        _NC_CACHE = _build_program()
    return _NC_CACHE


def _prep_in_maps(x1, x2, Wq, Wk):
    x1 = np.asarray(x1, np.float32)
    x2 = np.asarray(x2, np.float32)
    Wq = np.asarray(Wq, np.float32)
    Wk = np.asarray(Wk, np.float32)

    # R[c, h] = sum_j Wk[h*128+j, c] q[h*128+j] / sqrt(128)
    q = (Wq @ x1) * INV_SQRT_K                                  # [2048]
    R = np.einsum("hj,hjc->ch", q.reshape(H, J), Wk.reshape(H, J, C))
    rsb = np.ascontiguousarray(
        R.reshape(CH, 128, H).transpose(1, 0, 2)
    ).astype(_BF_NP)                                            # [128, 16, 16]

    in_maps = []
    for c in range(NCORES):
        shard = x2[c * S_LOC : (c + 1) * S_LOC]                 # [2048, 2048]
        # packed partition-major: x2t[p, cc, s] = shard.T[cc*128+p, s],
        #                         x2n[p, sc, c] = shard[sc*128+p, c]
        x2t_c = np.ascontiguousarray(
            shard.T.reshape(CH, 128, S_LOC).transpose(1, 0, 2)
        ).astype(_BF_NP)
        x2n_c = np.ascontiguousarray(
            shard.reshape(CH, 128, C).transpose(1, 0, 2)
        ).astype(_BF_NP)
        in_maps.append({"rsb": rsb, "x2t": x2t_c, "x2n": x2n_c})
    return in_maps


def _merge(results, Wv, Wo, bo):
    Wv = np.asarray(Wv, np.float32)
    Wo = np.asarray(Wo, np.float32)
    bo = np.asarray(bo, np.float32)
    t_tot = np.zeros((H, C), np.float64)
    l_tot = np.zeros(H, np.float64)
    for r in results:
        t_tot += r["tt"][:, :C].astype(np.float64)
        l_tot += r["tt"][:, C].astype(np.float64)
    tn = t_tot / l_tot[:, None]                                 # [16, 2048]
    u = np.einsum("hc,hjc->hj", tn, Wv.astype(np.float64).reshape(H, J, C))
    out = u.reshape(HJ) @ Wo.T.astype(np.float64) + bo.astype(np.float64)
    return out.astype(np.float32).reshape(1, ODIM)


def kernel(x1, x2, Wq, Wk, Wv, Wo, bo):
    nc = _get_nc()
    in_maps = _prep_in_maps(x1, x2, Wq, Wk)
    res = run_bass_kernel_spmd(nc, in_maps, list(range(NCORES)))
    return _merge(res.results, Wv, Wo, bo)


def run_traced(x1, x2, Wq, Wk, Wv, Wo, bo, **trace_kwargs):
    """Like kernel() but returns (output, BassKernelResults) with NTFF trace."""
    nc = _get_nc()
    in_maps = _prep_in_maps(x1, x2, Wq, Wk)
    res = run_bass_kernel_spmd(
        nc, in_maps, list(range(NCORES)), trace=True, **trace_kwargs
    )
    return _merge(res.results, Wv, Wo, bo), res



# revision 12
# speedup vs baseline: 1.5684x; 1.5684x over previous
"""Trainium2 Bass kernel for nn_CrossAttention_14207751815513.

Single-query cross-attention:
    q = x1 @ Wq.T                 (one query per head)
    k = x2 @ Wk.T ; v = x2 @ Wv.T
    attn_h = softmax(q_h . k_h / sqrt(128))
    out = concat_h(attn_h @ v_h) @ Wo.T + bo

Because there is exactly ONE query, the K and V projections collapse
algebraically (associativity):
    scores_h = x2 @ r_h,  r_h = Wk_h.T q_h / sqrt(128)   -- no k materialization
    out_h    = Wv_h @ (x2.T p_h) / l_h                   -- no v materialization
with p = exp(scores + EXP_BIAS) (the constant bias cancels in t/l) and
l_h = sum_s p_h[s].

Sharding: the sequence dim (16384) is split across the 8 NeuronCores
(2048 rows each).  All O(1)-in-S work (q, R, Wv matvec, Wo + bias) is
host-side glue; the O(S*C) work runs on device.

fp8 design (vs the earlier bf16 version: half the HBM bytes, double the
PE rate):
  - x2 is shipped in BOTH orientations as fp8e4 (e4m3): x2te [p, cc, s]
    (transposed, c-on-partitions) and x2n [p, sc, c].  8MB/core total.
  - All matmuls use fp8e4 DoubleRow perf mode: 2 k-tiles (256-deep
    contraction) per instruction, 0.5 cycles/row.
  - e4m3's 3 mantissa bits are too coarse for R (the folded query) and
    P (the exp'd scores); both use an UNSCALED two-term hi+lo split:
    v ~ e4(v) + e4(v - e4(v)).  Because the lo residual keeps the same
    scale, hi and lo matmuls accumulate into the SAME PSUM bank - the
    split costs only extra matmuls (PE is not the bottleneck), no extra
    combine ops.  Measured end-to-end rel err vs f32 reference: ~1.3e-2
    (gate: 2e-2).
  - R_hi/R_lo (2x16 cols per c-chunk) are embedded as columns
    2048:2080 of x2te, saving a DMA slot (only 8 HW-DGE slots exist).
  - exp is scalar.activation(Exp, scale=1/1024, bias=-2) straight from
    PSUM (scores are accumulated at 1024x scale for R's fp8 range); its
    accum_out gives the per-head softmax denominator l for free.
  - P_hi = e4(exp), P_lo = exp - P_hi (DVE tensor_sub), both transposed
    via PE into [p, sc, h] form for the phase-T lhsT.

Per-core device program:
  S  : scores[h, s] = sum_c (Rhi+Rlo)[c, h] x2t[c, s]  (8 cc-pairs x 2 terms
       x 4 banks, DoubleRow, shared PSUM accumulation)
  exp: P = exp(scores/1024 - 2), l = rowsum(P)         (ScalarE, accum_out)
  spl: Ph = e4(P); Pl = P - Ph                         (ScalarE copy + DVE sub)
  tr : Ph, Pl [16, 2048] -> PTh, PTl [128, 16 sb, 16]  (32 PE transposes)
  T  : t[h, c] = sum_s (PTh+PTl)[s, h] x2n[s, c]       (8 sc-pairs x 2 terms
       x 4 banks, DoubleRow, shared PSUM)

Output per core: tt [16, 2052] f32 = [t | 4 partial l columns].
Host merge: sum partials over cores, normalize by l, apply Wv per head,
then Wo + bo.

Sync-wait note: this backend disables DynamicDMA, so every HW-DGE DMA
lowers to a pseudo-direct DMA that supports at most ONE semaphore wait
("Too many sync wait commands" in walrus codegen otherwise).  The
program is therefore structured so no DMACopy ever needs two waits:
  - every streamed tile is a fresh buffer (unique pool tag, no reuse)
    so stream DMAs carry no WAR/WAW waits;
  - the program issues exactly 8 DMAs total (the 8 HW-DGE semaphore
    slots are assigned globally round-robin across both rings), so no
    DMA ever carries a slot-recycle wait on top of a RAW wait: 3 x2te
    chunks + 1 output on one split, 4 x2n chunks on the other, spread
    over the SP and Act rings so ~4MB streams per ring with x2te
    prioritized (phase S gates everything downstream);
  - the end-of-context Drain gets a sem wait for every proc the SP
    engine hasn't directly observed (the wait clock is not
    transitive), so an epilogue of single-dep SP nops makes SP
    observe each DMA and each engine's last instruction first.
"""

import sys

for _p in ("/root/.axon_site/_ro/trn_rl_repo", "/opt/trn_rl_repo"):
    if _p not in sys.path:
        sys.path.append(_p)

import numpy as np
import ml_dtypes

import concourse.bass as bass
import concourse.tile as tile
from concourse import mybir
from concourse.bass_utils import run_bass_kernel_spmd
from concourse.tile_rust import add_dep_helper

NCORES = 8
S_FULL = 16384
C = 2048           # input feature dim (both x1 and x2)
H = 16             # heads
J = 128            # head dim (K_DIM == V_DIM == 128)
HJ = H * J         # 2048
ODIM = 512
S_LOC = S_FULL // NCORES   # 2048 sequence rows per core

E4 = mybir.dt.float8e4
BF = mybir.dt.bfloat16
F32 = mybir.dt.float32
INV_SQRT_K = 1.0 / float(np.sqrt(128.0))
SCALE_R = 1024.0           # R is scaled up into e4m3's normal range
EXP_BIAS = -2.0            # keeps max p ~50 << e4m3 max 240 (cancels in t/l)

NB = 512                   # PSUM bank free-dim (f32 columns)
CH = C // 128              # 16 chunks of 128 along any 2048 dim
CE = C + 2 * H             # x2te free size: 2048 s-cols + 16 R_hi + 16 R_lo

_FP8_NP = ml_dtypes.float8_e4m3


def _build_program() -> bass.Bass:
    nc = bass.Bass()
    # Register EXP_BIAS as a const AP (mirrors Bass.__init__'s builtin 0.0 /
    # 1.0 consts): memset in the preamble, read with no dep edge, so the exp
    # activation keeps a single sync wait (its PE RAW).
    _bias_sb = nc.alloc_sbuf_tensor(f"const-f32-bias", [J, 1], F32)
    nc.gpsimd.memset(_bias_sb.ap(), EXP_BIAS)
    nc.const_aps.aps[(F32, EXP_BIAS)] = _bias_sb.ap()
    # Packed partition-major on the host ([p, chunk, col]) so a multi-chunk
    # stream DMA folds to ONE contiguous descriptor per partition.
    t_in = {
        "x2te": nc.dram_tensor("x2te", [J, CH, CE], E4, kind="ExternalInput"),
        "x2n": nc.dram_tensor("x2n", [J, CH, C], E4, kind="ExternalInput"),
    }
    t_out = {
        "tt": nc.dram_tensor("tt", [H, C + 4], F32, kind="ExternalOutput"),
    }

    x2te_v = t_in["x2te"][:, :, :]
    x2n_v = t_in["x2n"][:, :, :]
    tt_out = t_out["tt"][:, :]

    DR = mybir.MatmulPerfMode.DoubleRow

    with tile.TileContext(nc) as tc:
        with (
            tc.tile_pool(name="singles", bufs=1) as singles,
            tc.tile_pool(name="sa", bufs=1) as sa,
            tc.tile_pool(name="sb", bufs=1) as sbp,
            tc.tile_pool(name="psbig", bufs=4, space="PSUM") as psbig,
            tc.tile_pool(name="pstr", bufs=4, space="PSUM") as pstr,
        ):
            ep_targets = []  # one representative instruction per proc

            # ---- issue every stream DMA up front (both rings) --------------
            # SP ring: x2te cc0-1 (early matmul start), cc2-7, x2n sc0-5,
            #          sc14-15.  Act ring: x2te cc8-15, x2n sc6-11, sc12-13,
            #          then the output.  x2te leads both FIFOs since phase S
            #          gates exp -> PT -> phase T.
            xtr_a = sa.tile([J, 2, CE], E4, tag="xtr_a")
            ep_targets.append(nc.sync.dma_start(out=xtr_a, in_=x2te_v[:, 0:2, :]))
            xtr_b = sa.tile([J, 6, CE], E4, tag="xtr_b")
            ep_targets.append(nc.sync.dma_start(out=xtr_b, in_=x2te_v[:, 2:8, :]))
            xtr_c = sa.tile([J, 8, CE], E4, tag="xtr_c")
            ep_targets.append(nc.scalar.dma_start(out=xtr_c, in_=x2te_v[:, 8:16, :]))
            xn_a = sbp.tile([J, 6, C], E4, tag="xn_a")
            ep_targets.append(nc.sync.dma_start(out=xn_a, in_=x2n_v[:, 0:6, :]))
            xn_b = sbp.tile([J, 6, C], E4, tag="xn_b")
            ep_targets.append(nc.scalar.dma_start(out=xn_b, in_=x2n_v[:, 6:12, :]))
            xn_c = sbp.tile([J, 2, C], E4, tag="xn_c")
            ep_targets.append(nc.scalar.dma_start(out=xn_c, in_=x2n_v[:, 12:14, :]))
            xn_d = sbp.tile([J, 2, C], E4, tag="xn_d")
            ep_targets.append(nc.sync.dma_start(out=xn_d, in_=x2n_v[:, 14:16, :]))

            ident16 = singles.tile([H, H], BF)
            nc.gpsimd.memset(ident16, 0.0)
            # out[x, y] = (x - y) != 0 ? 0.0 : 1.0
            i_pool = nc.gpsimd.affine_select(
                out=ident16,
                in_=ident16,
                compare_op=mybir.AluOpType.not_equal,
                fill=1.0,
                base=0,
                pattern=[[-1, H]],
                channel_multiplier=1,
            )

            # ---- phase S: scores = sum_c (Rhi+Rlo).T x2t  (DoubleRow) ------
            ps = [
                psbig.tile([H, NB], F32, tag="big", name=f"ps_s{m}")
                for m in range(4)
            ]

            def xtr_tile(j):  # cc pair j -> (tile, local pair offset)
                if j == 0:
                    return xtr_a, 0
                if j <= 3:
                    return xtr_b, 2 * (j - 1)
                return xtr_c, 2 * (j - 4)

            for j in range(8):
                xt, o = xtr_tile(j)
                for hl in range(2):  # 0: R_hi cols, 1: R_lo cols
                    wc = C + H * hl
                    for m in range(4):
                        nc.tensor.matmul(
                            ps[m][:H, :],
                            lhsT=xt[:, o : o + 2, wc : wc + H],
                            rhs=xt[:, o : o + 2, m * NB : (m + 1) * NB],
                            start=(j == 0 and hl == 0),
                            stop=(j == 7 and hl == 1),
                            perf_mode=DR,
                        )

            # ---- exp (+ free l via accum_out) ------------------------------
            # P is kept in bf16 (8 mantissa bits); the fp8 hi+lo split is
            # formed AFTER the transpose, during the PSUM->SBUF copies.
            Psb = singles.tile([H, S_LOC], BF)
            tt_sb = singles.tile([H, C + 4], F32)
            for m in range(4):
                cols = slice(m * NB, (m + 1) * NB)
                nc.scalar.activation(
                    out=Psb[:, cols],
                    in_=ps[m][:H, :],
                    func=mybir.ActivationFunctionType.Exp,
                    scale=1.0 / SCALE_R,
                    bias=EXP_BIAS,
                    accum_out=tt_sb[:, C + m : C + m + 1],
                )

            # ---- transpose P (bf16), split into PTh + PTl (e4m3) -----------
            PTh = singles.tile([J, CH, H], E4)
            PTl = singles.tile([J, CH, H], E4)
            i_dve = i_sc = None
            for sb in range(CH):
                blk = slice(sb * J, (sb + 1) * J)
                pst = pstr.tile([J, H], BF, tag="tr", name=f"tr{sb}")
                nc.tensor.transpose(pst, Psb[:, blk], ident16)
                # hi copy and lo residual both on DVE: the sub's deps are
                # then same-engine (elided), keeping every wait count at 1.
                i_sc = nc.vector.tensor_copy(out=PTh[:, sb, :], in_=pst)
                i_dve = nc.vector.tensor_sub(
                    out=PTl[:, sb, :], in0=pst, in1=PTh[:, sb, :]
                )

            # ---- phase T: t = sum_s (PTh+PTl).T x2n  (DoubleRow) -----------
            pt = [
                psbig.tile([H, NB], F32, tag="big", name=f"ps_t{m}")
                for m in range(4)
            ]

            def xn_tile(j):  # sc pair j -> (tile, local pair offset)
                if j < 3:
                    return xn_a, 2 * j
                if j < 6:
                    return xn_b, 2 * (j - 3)
                if j == 6:
                    return xn_c, 0
                return xn_d, 0

            i_pe = None
            for idx, j in enumerate(range(8)):
                xn, o = xn_tile(j)
                for hl in range(2):
                    PT_ = PTh if hl == 0 else PTl
                    for m in range(4):
                        i_pe = nc.tensor.matmul(
                            pt[m][:H, :],
                            lhsT=PT_[:, 2 * j : 2 * j + 2, :],
                            rhs=xn[:, o : o + 2, m * NB : (m + 1) * NB],
                            start=(idx == 0 and hl == 0),
                            stop=(idx == 7 and hl == 1),
                            perf_mode=DR,
                        )

            i_copies = []
            for m in range(4):
                i_copies.append(
                    nc.scalar.copy(
                        out=tt_sb[:, m * NB : (m + 1) * NB], in_=pt[m][:H, :]
                    )
                )
            i_out = nc.scalar.dma_start(out=tt_out, in_=tt_sb)

            # ---- drain-funnel epilogue (see sync-wait note above) ----------
            ep_targets += [i_pool, i_dve, i_sc, i_pe, *i_copies, i_out]
            for t in ep_targets:
                n = nc.sync.nop(nofuse=True, hint="dep")
                add_dep_helper(n.ins, t.ins, reason="drain-funnel")

    return nc


_NC_CACHE = None


def _get_nc() -> bass.Bass:
    global _NC_CACHE
    if _NC_CACHE is None:
        _NC_CACHE = _build_program()
    return _NC_CACHE


def _prep_in_maps(x1, x2, Wq, Wk):
    x1 = np.asarray(x1, np.float32)
    x2 = np.asarray(x2, np.float32)
    Wq = np.asarray(Wq, np.float32)
    Wk = np.asarray(Wk, np.float32)

    # R[c, h] = sum_j Wk[h*128+j, c] q[h*128+j] / sqrt(128), scaled by 1024
    # and split into unscaled e4m3 hi+lo terms.
    q = (Wq @ x1) * INV_SQRT_K                                  # [2048]
    R = np.einsum("hj,hjc->ch", q.reshape(H, J), Wk.reshape(H, J, C))
    Rs = np.clip(R * SCALE_R, -240.0, 240.0).astype(np.float32)
    Rhi = Rs.astype(_FP8_NP)
    Rlo = (Rs - Rhi.astype(np.float32)).astype(_FP8_NP)
    # [p, cc, h] with c = cc*128 + p
    Rhi_p = np.ascontiguousarray(Rhi.reshape(CH, J, H).transpose(1, 0, 2))
    Rlo_p = np.ascontiguousarray(Rlo.reshape(CH, J, H).transpose(1, 0, 2))

    in_maps = []
    for c in range(NCORES):
        shard = x2[c * S_LOC : (c + 1) * S_LOC]                 # [2048, 2048]
        # packed partition-major: x2te[p, cc, s] = shard.T[cc*128+p, s],
        #                         x2n[p, sc, c] = shard[sc*128+p, c]
        x2te = np.empty((J, CH, CE), dtype=_FP8_NP)
        x2te[:, :, :C] = (
            shard.T.reshape(CH, J, S_LOC).transpose(1, 0, 2).astype(_FP8_NP)
        )
        x2te[:, :, C : C + H] = Rhi_p
        x2te[:, :, C + H :] = Rlo_p
        x2n_c = np.ascontiguousarray(
            shard.reshape(CH, J, C).transpose(1, 0, 2)
        ).astype(_FP8_NP)
        in_maps.append({"x2te": x2te, "x2n": x2n_c})
    return in_maps


def _merge(results, Wv, Wo, bo):
    Wv = np.asarray(Wv, np.float32)
    Wo = np.asarray(Wo, np.float32)
    bo = np.asarray(bo, np.float32)
    t_tot = np.zeros((H, C), np.float64)
    l_tot = np.zeros(H, np.float64)
    for r in results:
        t_tot += r["tt"][:, :C].astype(np.float64)
        l_tot += r["tt"][:, C:].astype(np.float64).sum(axis=1)
    tn = t_tot / l_tot[:, None]                                 # [16, 2048]
    u = np.einsum("hc,hjc->hj", tn, Wv.astype(np.float64).reshape(H, J, C))
    out = u.reshape(HJ) @ Wo.T.astype(np.float64) + bo.astype(np.float64)
    return out.astype(np.float32).reshape(1, ODIM)


def kernel(x1, x2, Wq, Wk, Wv, Wo, bo):
    nc = _get_nc()
    in_maps = _prep_in_maps(x1, x2, Wq, Wk)
    res = run_bass_kernel_spmd(nc, in_maps, list(range(NCORES)))
    return _merge(res.results, Wv, Wo, bo)


def run_traced(x1, x2, Wq, Wk, Wv, Wo, bo, **trace_kwargs):
    """Like kernel() but returns (output, BassKernelResults) with NTFF trace."""
    nc = _get_nc()
    in_maps = _prep_in_maps(x1, x2, Wq, Wk)
    res = run_bass_kernel_spmd(
        nc, in_maps, list(range(NCORES)), trace=True, **trace_kwargs
    )
    return _merge(res.results, Wv, Wo, bo), res


# revision 25
# speedup vs baseline: 1.8699x; 1.1922x over previous
"""Trainium2 Bass kernel for nn_CrossAttention_14207751815513.

Single-query cross-attention:
    q = x1 @ Wq.T                 (one query per head)
    k = x2 @ Wk.T ; v = x2 @ Wv.T
    attn_h = softmax(q_h . k_h / sqrt(128))
    out = concat_h(attn_h @ v_h) @ Wo.T + bo

Because there is exactly ONE query, the K and V projections collapse
algebraically (associativity):
    scores_h = x2 @ r_h,  r_h = Wk_h.T q_h / sqrt(128)   -- no k materialization
    out_h    = Wv_h @ (x2.T p_h) / l_h                   -- no v materialization
with p = exp(scores + EXP_BIAS) (the constant bias cancels in t/l) and
l_h = sum_s p_h[s].

Sharding: the sequence dim (16384) is split across the 8 NeuronCores
(2048 rows each).  All O(1)-in-S work (q, R, Wv matvec, Wo + bias) is
host-side glue; the O(S*C) work runs on device.

fp8 design (half the HBM bytes of bf16, double the PE rate):
  - x2 is shipped in BOTH orientations as fp8e4 (e4m3): x2te [p, cc, s]
    (transposed, c-on-partitions, with R embedded) and x2n [p, sc, c].
    ~8.4MB/core total; this stream is the roofline.
  - All big matmuls use fp8e4 DoubleRow perf mode: 2 k-tiles (256-deep
    contraction) per instruction, 0.5 cycles/row.
  - e4m3's 3 mantissa bits are too coarse for R (the folded query) and
    P (the exp'd scores); both use an UNSCALED two-term hi+lo split:
    v ~ e4(v) + e4(v - e4(v)).  Measured end-to-end rel err vs the f32
    reference: ~1.3e-2 (gate: 2e-2).
  - The hi/lo terms are packed side by side in the STATIONARY free dim
    (lhsT [128, 2, 32]), so one DoubleRow matmul emits [32, 512]: rows
    0:16 are the hi partial, rows 16:32 the lo partial.  Matmul cost
    scales only with streamed columns, so the split is FREE on the PE
    (32 matmuls per phase instead of 128).
  - Phase S rows are merged (hi+lo) before exp by a tiny f32r matmul
    with a stacked identity [I16; I16]; phase T rows are merged on the
    host.
  - R_hi/R_lo (2x16 cols per c-chunk) ride as columns 2048:2080 of
    x2te, saving a DMA slot (only 8 HW-DGE slots exist).
  - exp is scalar.activation(Exp, scale=1/1024, bias=-2) straight from
    the merged PSUM (scores carry a 1024x scale for R's fp8 range); its
    accum_out emits the per-head softmax denominator l for free.
  - P: exp writes bf16; after the PE transpose, PTh = e4(PT) (DVE copy)
    and PTl = PT - PTh (DVE sub) land side by side in PT[:, sb, 0:32].

Per-core device program:
  S  : scores32[hi|lo, s] = sum_c [Rhi|Rlo][c, :] x2t[c, s]   (8 cc-pairs
       x 4 banks, DoubleRow, [32, 512] PSUM banks)
  mrg: scores[h, s] = scores32[h, s] + scores32[16+h, s]      (DVE copy to
       SBUF + f32r identity matmul)
  exp: P = exp(scores/1024 - 2), l = rowsum(P)                (ScalarE)
  tr : P [16, 2048] -> PT [128, 16 sb, 16]; split hi/lo       (PE + DVE)
  T  : t32[hi|lo, c] = sum_s [PTh|PTl][s, :] x2n[s, c]        (8 sc-pairs
       x 4 banks, DoubleRow)
Outputs per core: tt [32, 2048] f32 (hi/lo partials), tl [16, 4] f32
(l partials).  Host: t = tt[:16] + tt[16:], l = tl.sum, normalize,
apply Wv, Wo + bo.

Sync-wait note: this backend disables DynamicDMA, so every HW-DGE DMA
lowers to a pseudo-direct DMA that supports at most ONE semaphore wait
("Too many sync wait commands" in walrus codegen otherwise), and the
Activation queue also supports only ONE wait per instruction.  The
program is therefore structured so no DMA or Activation ever needs two:
  - every streamed tile is a fresh buffer (unique pool tag, no reuse)
    so stream DMAs carry no WAR/WAW waits;
  - exactly 8 HW-DGE DMAs are issued (the 8 HW-DGE semaphore slots are
    assigned globally round-robin across both rings): 3 x2te chunks +
    5 x2n chunks, spread over the SP and Act rings with x2te leading
    both FIFOs (phase S gates everything downstream);
  - the two output DMAs go через the GpSimd SWDGE queue instead, each
    waiting on a single engine (DVE for tt, ScalarE for tl);
  - EXP_BIAS is registered as a const AP (mirroring the builtin consts)
    so the exp activation carries only its PE RAW wait;
  - the end-of-context Drain gets a sem wait for every proc the SP
    engine hasn't directly observed (the wait clock is not
    transitive), so an epilogue of single-dep SP nops makes SP
    observe each DMA and each engine's last instruction first.
"""

import sys

for _p in ("/root/.axon_site/_ro/trn_rl_repo", "/opt/trn_rl_repo"):
    if _p not in sys.path:
        sys.path.append(_p)

import numpy as np
import ml_dtypes

import concourse.bass as bass
import concourse.tile as tile
from concourse import mybir
from concourse.bass_utils import run_bass_kernel_spmd
from concourse.tile_rust import add_dep_helper

NCORES = 8
S_FULL = 16384
C = 2048           # input feature dim (both x1 and x2)
H = 16             # heads
H2 = 2 * H         # hi|lo packed output rows
J = 128            # head dim (K_DIM == V_DIM == 128)
HJ = H * J         # 2048
ODIM = 512
S_LOC = S_FULL // NCORES   # 2048 sequence rows per core

E4 = mybir.dt.float8e4
BF = mybir.dt.bfloat16
F32 = mybir.dt.float32
F32R = mybir.dt.float32r
INV_SQRT_K = 1.0 / float(np.sqrt(128.0))
SCALE_R = 1024.0           # R is scaled up into e4m3's normal range
EXP_BIAS = -2.0            # keeps max p ~50 << e4m3 max 240 (cancels in t/l)

NB = 512                   # PSUM bank free-dim (f32 columns)
CH = C // 128              # 16 chunks of 128 along any 2048 dim
CE = C + 2 * H             # x2te free size: 2048 s-cols + 16 R_hi + 16 R_lo

_FP8_NP = ml_dtypes.float8_e4m3


def _build_program() -> bass.Bass:
    nc = bass.Bass()
    # Register EXP_BIAS as a const AP (mirrors Bass.__init__'s builtin 0.0 /
    # 1.0 consts): memset in the preamble, read with no dep edge, so the exp
    # activation keeps a single sync wait (its PE RAW).
    _bias_sb = nc.alloc_sbuf_tensor("const-f32-bias", [J, 1], F32)
    nc.gpsimd.memset(_bias_sb.ap(), EXP_BIAS)
    nc.const_aps.aps[(F32, EXP_BIAS)] = _bias_sb.ap()

    t_in = {
        "x2te": nc.dram_tensor("x2te", [J, CH, CE], E4, kind="ExternalInput"),
        "x2n": nc.dram_tensor("x2n", [J, CH, C], E4, kind="ExternalInput"),
    }
    t_out = {
        "tt": nc.dram_tensor("tt", [H2, C], F32, kind="ExternalOutput"),
        "tl": nc.dram_tensor("tl", [H, 4], F32, kind="ExternalOutput"),
    }

    x2te_v = t_in["x2te"][:, :, :]
    x2n_v = t_in["x2n"][:, :, :]

    DR = mybir.MatmulPerfMode.DoubleRow

    with tile.TileContext(nc) as tc:
        with (
            tc.tile_pool(name="singles", bufs=1) as singles,
            tc.tile_pool(name="sa", bufs=1) as sa,
            tc.tile_pool(name="sb", bufs=1) as sbp,
            tc.tile_pool(name="psbig", bufs=4, space="PSUM") as psbig,
            tc.tile_pool(name="pstr", bufs=4, space="PSUM") as pstr,
        ):
            ep_targets = []  # one representative instruction per proc

            # ---- issue every stream DMA up front (both rings) --------------
            # SP ring: x2te cc0-1 (early matmul start), cc2-7, then x2n
            # pairs 0, 3-4, 7.  Act ring: x2te cc8-15, x2n pairs 1-2, 5-6.
            # x2te leads both FIFOs since phase S gates exp -> PT -> phase T;
            # x2n chunks then arrive spread so phase T tracks the stream.
            xtr_a = sa.tile([J, 2, CE], E4, tag="xtr_a")
            ep_targets.append(nc.sync.dma_start(out=xtr_a, in_=x2te_v[:, 0:2, :]))
            xtr_b = sa.tile([J, 6, CE], E4, tag="xtr_b")
            ep_targets.append(nc.sync.dma_start(out=xtr_b, in_=x2te_v[:, 2:8, :]))
            xtr_c = sa.tile([J, 8, CE], E4, tag="xtr_c")
            ep_targets.append(nc.scalar.dma_start(out=xtr_c, in_=x2te_v[:, 8:16, :]))
            xn_a = sbp.tile([J, 2, C], E4, tag="xn_a")
            ep_targets.append(nc.sync.dma_start(out=xn_a, in_=x2n_v[:, 0:2, :]))
            xn_b = sbp.tile([J, 4, C], E4, tag="xn_b")
            ep_targets.append(nc.scalar.dma_start(out=xn_b, in_=x2n_v[:, 2:6, :]))
            xn_c = sbp.tile([J, 4, C], E4, tag="xn_c")
            ep_targets.append(nc.sync.dma_start(out=xn_c, in_=x2n_v[:, 6:10, :]))
            xn_d = sbp.tile([J, 4, C], E4, tag="xn_d")
            ep_targets.append(nc.scalar.dma_start(out=xn_d, in_=x2n_v[:, 10:14, :]))
            xn_e = sbp.tile([J, 2, C], E4, tag="xn_e")
            ep_targets.append(nc.sync.dma_start(out=xn_e, in_=x2n_v[:, 14:16, :]))

            ident16 = singles.tile([H, H], BF)
            nc.gpsimd.memset(ident16, 0.0)
            # out[x, y] = (x - y) != 0 ? keep : 1.0
            nc.gpsimd.affine_select(
                out=ident16,
                in_=ident16,
                compare_op=mybir.AluOpType.not_equal,
                fill=1.0,
                base=0,
                pattern=[[-1, H]],
                channel_multiplier=1,
            )
            # stacked identity [I16; I16] for the hi+lo score merge
            ident32 = singles.tile([H2, H], F32)
            nc.gpsimd.memset(ident32, 0.0)
            nc.gpsimd.affine_select(
                out=ident32,
                in_=ident32,
                compare_op=mybir.AluOpType.not_equal,
                fill=1.0,
                base=0,
                pattern=[[-1, H]],
                channel_multiplier=1,
            )
            i_pool = nc.gpsimd.affine_select(
                out=ident32,
                in_=ident32,
                compare_op=mybir.AluOpType.not_equal,
                fill=1.0,
                base=-H,
                pattern=[[-1, H]],
                channel_multiplier=1,
            )
            # bounce through the DVE so the first merge matmul's deps are a
            # single DVE sem (the LdWeights slot only fits one sync wait)
            ident32v = singles.tile([H2, H], F32R)
            nc.vector.tensor_copy(out=ident32v, in_=ident32)


            # ---- phase S: scores32 = [Rhi|Rlo].T @ x2t  (DoubleRow) --------
            ps = [
                psbig.tile([H2, NB], F32, tag="big", name=f"ps_s{m}")
                for m in range(4)
            ]

            def xtr_tile(j):  # cc pair j -> (tile, local pair offset)
                if j == 0:
                    return xtr_a, 0
                if j <= 3:
                    return xtr_b, 2 * (j - 1)
                return xtr_c, 2 * (j - 4)

            for j in range(8):
                xt, o = xtr_tile(j)
                for m in range(4):
                    nc.tensor.matmul(
                        ps[m][:H2, :],
                        lhsT=xt[:, o : o + 2, C : C + H2],
                        rhs=xt[:, o : o + 2, m * NB : (m + 1) * NB],
                        start=(j == 0),
                        stop=(j == 7),
                        perf_mode=DR,
                    )

            # ---- merge hi+lo rows, exp (+ free l via accum_out) ------------
            ssb = singles.tile([H2, S_LOC], F32R)
            for m in range(4):
                nc.vector.tensor_copy(
                    out=ssb[:, m * NB : (m + 1) * NB], in_=ps[m][:H2, :]
                )
            mg = [
                psbig.tile([H2, NB], F32, tag="big", name=f"ps_mg{m}")
                for m in range(4)
            ]
            for m in range(4):
                nc.tensor.matmul(
                    mg[m][:H, :],
                    lhsT=ident32v[:, :],
                    rhs=ssb[:, m * NB : (m + 1) * NB],
                    start=True,
                    stop=True,
                )
            Psb = singles.tile([H, S_LOC], BF)
            tl_sb = singles.tile([H, 4], F32)
            i_exp = None
            for m in range(4):
                i_exp = nc.scalar.activation(
                    out=Psb[:, m * NB : (m + 1) * NB],
                    in_=mg[m][:H, :],
                    func=mybir.ActivationFunctionType.Exp,
                    scale=1.0 / SCALE_R,
                    bias=EXP_BIAS,
                    accum_out=tl_sb[:, m : m + 1],
                )

            # ---- transpose P (bf16), split into PT = [hi | lo] (e4m3) ------
            PT = singles.tile([J, CH, H2], E4)
            i_dve = None
            for sb in range(CH):
                blk = slice(sb * J, (sb + 1) * J)
                pst = pstr.tile([J, H], BF, tag="tr", name=f"tr{sb}")
                nc.tensor.transpose(pst, Psb[:, blk], ident16)
                nc.vector.tensor_copy(out=PT[:, sb, 0:H], in_=pst)
                i_dve = nc.vector.tensor_sub(
                    out=PT[:, sb, H:H2], in0=pst, in1=PT[:, sb, 0:H]
                )

            # ---- phase T: t32 = [PTh|PTl].T @ x2n  (DoubleRow) -------------
            pt = [
                psbig.tile([H2, NB], F32, tag="big", name=f"ps_t{m}")
                for m in range(4)
            ]

            def xn_tile(j):  # sc pair j -> (tile, local pair offset)
                if j == 0:
                    return xn_a, 0
                if j <= 2:
                    return xn_b, 2 * (j - 1)
                if j <= 4:
                    return xn_c, 2 * (j - 3)
                if j <= 6:
                    return xn_d, 2 * (j - 5)
                return xn_e, 0

            i_pe = None
            for j in range(8):
                xn, o = xn_tile(j)
                for m in range(4):
                    i_pe = nc.tensor.matmul(
                        pt[m][:H2, :],
                        lhsT=PT[:, 2 * j : 2 * j + 2, :],
                        rhs=xn[:, o : o + 2, m * NB : (m + 1) * NB],
                        start=(j == 0),
                        stop=(j == 7),
                        perf_mode=DR,
                    )

            tt_sb = singles.tile([H2, C], F32)
            i_tcopies = []
            for m in range(4):
                i_tcopies.append(
                    nc.vector.tensor_copy(
                        out=tt_sb[:, m * NB : (m + 1) * NB], in_=pt[m][:H2, :]
                    )
                )
            # output DMAs on the GpSimd SWDGE queue: each waits on exactly one
            # engine (ScalarE for tl, DVE for tt)
            i_out_l = nc.gpsimd.dma_start(out=t_out["tl"][:, :], in_=tl_sb)
            i_out_t = nc.gpsimd.dma_start(out=t_out["tt"][:, :], in_=tt_sb)
            # trailing Pool nop: its engine-sem value covers the SWDGE DMA
            # instructions, so the funnel can make SP observe Pool fully
            i_gp_last = nc.gpsimd.nop(nofuse=True, hint="dep")

            # ---- drain-funnel epilogue (see sync-wait note above) ----------
            ep_targets += [
                i_pool, i_exp, i_dve, i_pe, *i_tcopies,
                i_out_l, i_out_t, i_gp_last,
            ]
            for t in ep_targets:
                n = nc.sync.nop(nofuse=True, hint="dep")
                add_dep_helper(n.ins, t.ins, reason="drain-funnel")

    return nc


_NC_CACHE = None


def _get_nc() -> bass.Bass:
    global _NC_CACHE
    if _NC_CACHE is None:
        _NC_CACHE = _build_program()
    return _NC_CACHE


def _prep_in_maps(x1, x2, Wq, Wk):
    x1 = np.asarray(x1, np.float32)
    x2 = np.asarray(x2, np.float32)
    Wq = np.asarray(Wq, np.float32)
    Wk = np.asarray(Wk, np.float32)

    # R[c, h] = sum_j Wk[h*128+j, c] q[h*128+j] / sqrt(128), scaled by 1024
    # and split into unscaled e4m3 hi+lo terms.
    q = (Wq @ x1) * INV_SQRT_K                                  # [2048]
    R = np.einsum("hj,hjc->ch", q.reshape(H, J), Wk.reshape(H, J, C))
    Rs = np.clip(R * SCALE_R, -240.0, 240.0).astype(np.float32)
    Rhi = Rs.astype(_FP8_NP)
    Rlo = (Rs - Rhi.astype(np.float32)).astype(_FP8_NP)
    # [p, cc, h] with c = cc*128 + p
    Rhi_p = np.ascontiguousarray(Rhi.reshape(CH, J, H).transpose(1, 0, 2))
    Rlo_p = np.ascontiguousarray(Rlo.reshape(CH, J, H).transpose(1, 0, 2))

    in_maps = []
    for c in range(NCORES):
        shard = x2[c * S_LOC : (c + 1) * S_LOC]                 # [2048, 2048]
        # packed partition-major: x2te[p, cc, s] = shard.T[cc*128+p, s],
        #                         x2n[p, sc, c] = shard[sc*128+p, c]
        x2te = np.empty((J, CH, CE), dtype=_FP8_NP)
        x2te[:, :, :C] = (
            shard.T.reshape(CH, J, S_LOC).transpose(1, 0, 2).astype(_FP8_NP)
        )
        x2te[:, :, C : C + H] = Rhi_p
        x2te[:, :, C + H :] = Rlo_p
        x2n_c = np.ascontiguousarray(
            shard.reshape(CH, J, C).transpose(1, 0, 2)
        ).astype(_FP8_NP)
        in_maps.append({"x2te": x2te, "x2n": x2n_c})
    return in_maps


def _merge(results, Wv, Wo, bo):
    Wv = np.asarray(Wv, np.float32)
    Wo = np.asarray(Wo, np.float32)
    bo = np.asarray(bo, np.float32)
    t_tot = np.zeros((H, C), np.float64)
    l_tot = np.zeros(H, np.float64)
    for r in results:
        tt = r["tt"].astype(np.float64)
        t_tot += tt[:H] + tt[H:]
        l_tot += r["tl"].astype(np.float64).sum(axis=1)
    tn = t_tot / l_tot[:, None]                                 # [16, 2048]
    u = np.einsum("hc,hjc->hj", tn, Wv.astype(np.float64).reshape(H, J, C))
    out = u.reshape(HJ) @ Wo.T.astype(np.float64) + bo.astype(np.float64)
    return out.astype(np.float32).reshape(1, ODIM)


def kernel(x1, x2, Wq, Wk, Wv, Wo, bo):
    nc = _get_nc()
    in_maps = _prep_in_maps(x1, x2, Wq, Wk)
    res = run_bass_kernel_spmd(nc, in_maps, list(range(NCORES)))
    return _merge(res.results, Wv, Wo, bo)


def run_traced(x1, x2, Wq, Wk, Wv, Wo, bo, **trace_kwargs):
    """Like kernel() but returns (output, BassKernelResults) with NTFF trace."""
    nc = _get_nc()
    in_maps = _prep_in_maps(x1, x2, Wq, Wk)
    res = run_bass_kernel_spmd(
        nc, in_maps, list(range(NCORES)), trace=True, **trace_kwargs
    )
    return _merge(res.results, Wv, Wo, bo), res
